# revision 27
# baseline (speedup 1.0000x reference)
"""Trainium2 Bass kernel for nn_BAKT_32006096290477 (dense transformer,
BAKT-style attention; B=32, S=512, D=512, H=8, L=4, F=2048).

kernel(**inputs) takes the FULL unsharded inputs (as produced by
reference.setup_inputs), shards data-parallel over batch across 8
NeuronCores (4 sequences per core), compiles+runs a Bass/Tile kernel via
run_bass_kernel_spmd, and gathers the full (B, S, D) float32 output.

On-device layout: feature-major [D, T] activations (features on SBUF
partitions, tokens on the free axis); per-layer weights stationary.
The per-layer work is emitted as a 4-stage software pipeline skewed
across the 4 local sequences (S0 proj / S1 attention / S2 out+LN1 /
S3 FFN+LN2) so the TensorE stream always has independent matmuls
adjacent to every serial LayerNorm/softmax chain — this keeps the PE
HAM clock-gate warm and fills dependency stalls.
"""

import math
import sys
from contextlib import ExitStack

sys.path.insert(0, "/opt/trn_rl_repo")

import numpy as np
import orjson

import concourse.bass as bass
import concourse.tile as tile
from concourse import bass_utils, bass2jax, mybir
from concourse.vector_clock import ScopedClock

_CARRIER_OPCODE = "NoOp"


def _split_bir_multiwaits(bir_json: bytes) -> bytes:
    d = orjson.loads(bir_json)
    n_carriers = 0
    for fn in d.get("functions", []):
        for bb in fn.get("blocks", []):
            insts = bb.get("instructions", [])
            out = []
            for inst in insts:
                si = inst.get("sync_info") or {}
                waits = si.get("on_wait") or []
                if len(waits) > 1:
                    for k, w in enumerate(waits[:-1]):
                        out.append(
                            {
                                "debug": inst.get("debug", 0),
                                "engine": inst["engine"],
                                "ins": [],
                                "name": f"{inst['name']}-w{k}",
                                "opcode": _CARRIER_OPCODE,
                                "outs": [],
                                "sync_info": {"on_update": [], "on_wait": [w]},
                            }
                        )
                        n_carriers += 1
                    si["on_wait"] = [waits[-1]]
                out.append(inst)
            bb["instructions"] = out
    if n_carriers:
        print(f"[bass_compat] split {n_carriers} excess sync-waits onto NoOp carriers")
    return orjson.dumps(d)


_orig_compile = bass_utils.compile_bir_kernel


def _patched_compile(bir_json, tmpdir, neff_name="file.neff"):
    return _orig_compile(_split_bir_multiwaits(bir_json), tmpdir, neff_name=neff_name)


def _patched_drain_and_barrier(self, tick_clock, wait_clock):
    nc = self.nc
    drain_inst = nc.sync.drain()
    wait_clock.add_sem_waits(
        drain_inst.ins, ScopedClock({None: tick_clock.global_clock})
    )
    si = drain_inst.ins.sync_info
    if si is not None and len(si.on_wait) > 1:
        waits = list(si.on_wait)
        ups = list(si.on_update)
        drain_inst.ins.sync_info = mybir.SyncInfo(on_wait=[waits[0]], on_update=ups)
        for w in waits[1:]:
            d2 = nc.sync.drain()
            d2.ins.sync_info = mybir.SyncInfo(on_wait=[w], on_update=[])
    nc.all_engine_barrier()
    popped = nc._tile_sem_poison_stack.pop()
    assert popped is self._sem_poison
    nc.clear_and_free_semaphores(list(self.sems.allocated().values()))
    nc.all_engine_barrier()


def install():
    bass_utils.compile_bir_kernel = _patched_compile
    bass2jax.compile_bir_kernel = _patched_compile
    tile.TileContext._drain_and_barrier = _patched_drain_and_barrier
    # zero-egress container: keep NTFF/perfetto artifacts local
    bass_utils.upload_artifacts = lambda tmpdir: tmpdir


install()


def _install_ntff_hook():
    """Register the axon NTFF profile hook (the agent image's antenv lacks
    axon_hooks, so bass_utils' trace path would otherwise ImportError).
    Dev/profiling only — called when KT_TRACE=1."""
    import contextlib
    import ctypes
    import types

    import antenv

    if hasattr(antenv, "axon_hooks"):
        return
    so_path = "/opt/axon/libaxon_pjrt.so"
    lib = ctypes.CDLL(so_path)
    if not hasattr(lib, "axon_start_nrt_profile"):
        return
    lib.axon_start_nrt_profile.argtypes = [
        ctypes.POINTER(ctypes.c_int64),
        ctypes.c_size_t,
    ]
    lib.axon_start_nrt_profile.restype = ctypes.c_int64
    lib.axon_stop_nrt_profile.argtypes = [ctypes.c_char_p]
    lib.axon_stop_nrt_profile.restype = ctypes.c_int64

    @contextlib.contextmanager
    def _hook(output_dir, device_ids):
        import jax

        jax.devices()
        if device_ids:
            ids = (ctypes.c_int64 * len(device_ids))(*device_ids)
            rc = lib.axon_start_nrt_profile(ids, len(device_ids))
        else:
            rc = lib.axon_start_nrt_profile(None, 0)
        if rc != 0:
            raise RuntimeError(f"axon_start_nrt_profile rc={rc}")
        try:
            yield
        finally:
            n = lib.axon_stop_nrt_profile(str(output_dir).encode())
            print(f"profile: {n} file(s) written to {output_dir}", file=sys.stderr)

    mod = types.ModuleType("antenv.axon_hooks")
    mod._hook = _hook
    mod.get_axon_ntff_profile_hook = lambda: _hook
    mod.set_axon_ntff_profile_hook = lambda h: None
    sys.modules["antenv.axon_hooks"] = mod
    antenv.axon_hooks = mod


F32 = mybir.dt.float32
F32R = mybir.dt.float32r
BF16 = mybir.dt.bfloat16
AF = mybir.ActivationFunctionType
ALU = mybir.AluOpType
P = 128
DK = 64
EPS = 1e-5


class Cfg:
    def __init__(self, Bl, S, D, H, F, L):
        assert D % P == 0 and F % P == 0 and S % P == 0 and S >= 256 and S <= 512
        assert H * DK == D and H % 2 == 0
        self.Bl, self.S, self.D, self.H, self.F, self.L = Bl, S, D, H, F, L
        self.T = Bl * S
        self.DT = D // P   # feature tiles
        self.FT = F // P   # ff tiles
        self.SB = S // P   # key blocks per sequence


def build(cfg: Cfg, trivial_affine: bool):
    c = cfg
    nc = bass.Bass()

    dp = nc.declare_dram_parameter
    xT = dp("xT", [c.D, c.T], F32, isOutput=False)
    yT = dp("yT", [c.D, c.T], BF16, isOutput=False)
    cvec = dp("cvec", [1, c.T], F32, isOutput=False)
    wkT = dp("wkT", [c.L, c.D, c.D], BF16, isOutput=False)
    wvT = dp("wvT", [c.L, c.D, c.D], BF16, isOutput=False)
    woT = dp("woT", [c.L, c.D, c.D], BF16, isOutput=False)
    w1T = dp("w1T", [c.L, c.D, c.F], BF16, isOutput=False)
    w2T = dp("w2T", [c.L, c.F, c.D], BF16, isOutput=False)
    bkc = dp("bkc", [c.L, P, c.DT], F32, isOutput=False)
    bo2c = dp("bo2c", [c.L, P, c.DT], F32, isOutput=False)
    b1c = dp("b1c", [c.L, P, c.FT], F32, isOutput=False)
    b2c = dp("b2c", [c.L, P, c.DT], F32, isOutput=False)
    lnrow = dp("lnrow", [c.L, 1, 4 * c.D], F32, isOutput=False)  # g1,b1,g2,b2
    mtri = dp("mtri", [P, P], BF16, isOutput=False)  # [j,i] = 1.0 if j<i
    xoT = dp("xoT", [c.D, c.T], F32, isOutput=True)

    with tile.TileContext(nc) as tc, ExitStack() as _es:
        ep = _es.enter_context
        cst = ep(tc.tile_pool(name="cst", bufs=1))
        cst2 = ep(tc.tile_pool(name="cst2", bufs=2))    # per-layer bias consts
        xp = ep(tc.tile_pool(name="xp", bufs=4))        # x residual (f32r), per-dt
        xbp = ep(tc.tile_pool(name="xbp", bufs=1))      # bf16 x for kq rhs
        up = ep(tc.tile_pool(name="up", bufs=2))        # u tiles (f32r), per-e
        x1bp = ep(tc.tile_pool(name="x1b", bufs=2))     # bf16 x1 for FFN
        tmpp = ep(tc.tile_pool(name="tmpp", bufs=2))    # LN apply temp (f32)
        kqp = ep(tc.tile_pool(name="kqp", bufs=2))      # bf16 keys, per-e
        stgp = ep(tc.tile_pool(name="stgp", bufs=2))    # bf16 scaled queries
        vpp = ep(tc.tile_pool(name="vpp", bufs=1))      # bf16 v, persistent per (tt, par)
        yp = ep(tc.tile_pool(name="yp", bufs=1))        # bf16 y, persistent per (dt,b)
        ctxp = ep(tc.tile_pool(name="ctxp", bufs=2))    # bf16 ctx, per-dt
        ptp = ep(tc.tile_pool(name="ptp", bufs=3))      # bf16 exp(S^T) tiles
        rbp = ep(tc.tile_pool(name="rbp", bufs=2))      # f32 recip bcast [P,S]
        hp = ep(tc.tile_pool(name="hp", bufs=16))       # bf16 FFN hidden tiles
        sqp = ep(tc.tile_pool(name="sqp", bufs=2))      # bf16 squared tiles
        rows = ep(tc.tile_pool(name="rows", bufs=3))    # [1,S] f32 rows
        lnr = ep(tc.tile_pool(name="lnr", bufs=4))      # [1,S] f32r a/b rows
        rowb = ep(tc.tile_pool(name="rowb", bufs=2))    # [1,S] bf16 rows
        wsm = ep(tc.tile_pool(name="wsm", bufs=1))      # bf16 wk/wv/wo
        w1p = ep(tc.tile_pool(name="w1p", bufs=1))      # bf16 w1
        w2p = ep(tc.tile_pool(name="w2p", bufs=1))      # bf16 w2
        pmm = ep(tc.tile_pool(name="pmm", bufs=2, space="PSUM"))
        psc = ep(tc.tile_pool(name="psc", bufs=2, space="PSUM"))
        pctx = ep(tc.tile_pool(name="pctx", bufs=2, space="PSUM"))
        paux = ep(tc.tile_pool(name="paux", bufs=2, space="PSUM"))

        f32 = lambda ap: ap.bitcast(F32)

        # ---------------- constants ----------------
        ones_f = cst.tile([P, c.H], F32, tag="ones_f")
        nc.gpsimd.memset(ones_f[:], 1.0)
        zeros_f = cst.tile([P, 1], F32, tag="zeros_f")
        nc.gpsimd.memset(zeros_f[:], 0.0)
        ones_col = cst.tile([P, 1], F32R, tag="ones_col")
        nc.scalar.copy(out=ones_col[:], in_=ones_f[:, 0:1])
        onesb_col = cst.tile([P, 1], BF16, tag="onesb_col")
        nc.scalar.copy(out=onesb_col[:], in_=ones_f[:, 0:1])
        onesr_f = rows.tile([1, c.S], F32, tag="row")
        nc.gpsimd.memset(onesr_f[:], 1.0)
        ones_row = cst.tile([1, c.S], F32R, tag="ones_row")
        nc.scalar.copy(out=ones_row[:], in_=onesr_f[:])
        ones_row_b = cst.tile([1, P], BF16, tag="ones_row_b")
        nc.scalar.copy(out=ones_row_b[:], in_=onesr_f[:, 0:P])
        eps_t = cst.tile([1, 1], F32, tag="eps")
        nc.gpsimd.memset(eps_t[:], EPS)
        mtri_sb = cst.tile([P, P], BF16, tag="mtri")
        nc.sync.dma_start(out=mtri_sb[:], in_=mtri[:])
        crow = cst.tile([P, c.T], BF16, tag="crow")
        for ch in range(c.T // c.S):
            cv = rows.tile([1, c.S], F32R, tag="row")
            nc.sync.dma_start(out=cv[:], in_=cvec[:, ch * c.S:(ch + 1) * c.S].bitcast(F32R))
            pb = psc.tile([P, c.S], F32, tag="sc")
            nc.tensor.matmul(pb[:], ones_row[0:1, 0:P], cv[:], start=True, stop=True)
            nc.scalar.copy(out=crow[:, ch * c.S:(ch + 1) * c.S], in_=pb[:])

        # ------------- weight loading (progressive prefetch) -------------
        # Wt[l][group] holds layer-l weight tiles. DMAs for layer l+1 are
        # emitted at the program points where the bufs=1 tag slots free up
        # (the Sync DMA queue is in-order, so emission order must match the
        # order the WAR wait-conditions fire — else head-of-line blocking
        # starves the next layer's first matmuls).
        Wt = {l: {} for l in range(c.L)}

        def load_w(l, groups):
            d = Wt[l]
            for g in groups:
                if g == "wk":
                    d["wk"] = []
                    for dt in range(c.DT):
                        t = wsm.tile([P, c.D], BF16, tag=f"wk{dt}", name=f"wk{dt}")
                        nc.sync.dma_start(out=t[:], in_=wkT[l, dt * P:(dt + 1) * P, :])
                        d["wk"].append(t)
                elif g == "wv":
                    d["wv"] = []
                    for dt in range(c.DT):
                        t = wsm.tile([P, c.D], BF16, tag=f"wv{dt}", name=f"wv{dt}")
                        nc.sync.dma_start(out=t[:], in_=wvT[l, dt * P:(dt + 1) * P, :])
                        d["wv"].append(t)
                elif g == "wo":
                    d["wo"] = []
                    for dt in range(c.DT):
                        t = wsm.tile([P, c.D], BF16, tag=f"wo{dt}", name=f"wo{dt}")
                        nc.sync.dma_start(out=t[:], in_=woT[l, dt * P:(dt + 1) * P, :])
                        d["wo"].append(t)
                elif g == "w1":
                    d["w1"] = []
                    for dt in range(c.DT):
                        t = w1p.tile([P, c.F], BF16, tag=f"w1{dt}", name=f"w1{dt}")
                        nc.sync.dma_start(out=t[:], in_=w1T[l, dt * P:(dt + 1) * P, :])
                        d["w1"].append(t)
                elif g == "w2":
                    d["w2"] = []
                    for ft in range(c.FT):
                        t = w2p.tile([P, c.D], BF16, tag=f"w2{ft}", name=f"w2{ft}")
                        nc.sync.dma_start(out=t[:], in_=w2T[l, ft * P:(ft + 1) * P, :])
                        d["w2"].append(t)
                elif g == "bias":
                    t = cst2.tile([P, c.DT], F32, tag="bk", name="bk")
                    nc.sync.dma_start(out=t[:], in_=bkc[l])
                    d["bk"] = t
                    t = cst2.tile([P, c.DT], F32, tag="bo2", name="bo2")
                    nc.sync.dma_start(out=t[:], in_=bo2c[l])
                    d["bo2"] = t
                    t = cst2.tile([P, c.FT], F32, tag="b1", name="b1")
                    nc.sync.dma_start(out=t[:], in_=b1c[l])
                    d["b1"] = t
                    t = cst2.tile([P, c.DT], F32, tag="b2", name="b2")
                    nc.sync.dma_start(out=t[:], in_=b2c[l])
                    d["b2"] = t
                    if not trivial_affine:
                        t = cst2.tile([1, 4 * c.D], F32R, tag="ln", name="ln_t")
                        nc.sync.dma_start(out=t[:], in_=lnrow[l].bitcast(F32R))
                        d["ln"] = t

        # ------------- startup: first-needed tiles first -------------
        xt_w = [None] * c.Bl
        y_sb = [[None] * c.Bl for _ in range(c.DT)]
        # persistent v tiles (double-buffered manually); ones column written once
        vt_pers = [[None, None] for _ in range(c.SB)]
        for tt in range(c.SB):
            for par in range(2):
                vt = vpp.tile([P, c.H, DK + 1], BF16, tag=f"vp{tt}p{par}",
                              name=f"vp{tt}p{par}")
                nc.scalar.copy(
                    out=vt[:, :, DK:DK + 1],
                    in_=ones_f[:, :].rearrange("p (h o) -> p h o", o=1))
                vt_pers[tt][par] = vt

        def load_x(b):
            t = xp.tile([P, c.DT * c.S], F32R, tag="x", name="x")
            for dt in range(c.DT):
                nc.sync.dma_start(
                    out=t[:, dt * c.S:(dt + 1) * c.S],
                    in_=xT[dt * P:(dt + 1) * P, b * c.S:(b + 1) * c.S].bitcast(F32R))
            xt_w[b] = t

        def load_y(b):
            for dt in range(c.DT):
                t = yp.tile([P, c.S], BF16, tag=f"y{dt}b{b}", name=f"y{dt}b{b}")
                nc.sync.dma_start(
                    out=t[:], in_=yT[dt * P:(dt + 1) * P, b * c.S:(b + 1) * c.S])
                y_sb[dt][b] = t

        load_w(0, ["wk", "bias"])
        load_x(0)
        load_w(0, ["wv"])
        load_y(0)
        for b in range(1, c.Bl):
            load_x(b)
            load_y(b)
        load_w(0, ["wo", "w1", "w2"])

        # =========================== layers ===========================
        # Six pipeline stages per (layer, seq):
        #   0 s_proj   kq/v projections
        #   1 s_attn   attention (scores, softmax, ctx)
        #   2 s_outs   out-proj + residual + LN1 stats/row-chain
        #   3 s_ln1ap  LN1 broadcast + apply + bf16 casts for FFN
        #   4 s_ffn    FFN W1/W2 + residual + LN2 stats/row-chain
        #   5 s_ln2ap  LN2 broadcast + apply (+ output DMA on last layer)
        # Emitted with a global skew gstep = 5*l + si + b: every serial
        # row-chain has other (layer, seq) matmuls adjacent in each engine's
        # static stream, and layer boundaries overlap (layer l+1 s_proj
        # starts while layer l drains).
        kq_of, stg_of, vpl_of, ctx_of, u_of, x1b_of, ln_of = {}, {}, {}, {}, {}, {}, {}
        xb_pre = {}

        def ln_stats(u_t):
            """Feature-axis LN stats of u_t ([P, DT*S] f32r): returns
            (a_row, b_row) with A=rstd, B=-(mean)*rstd."""
            pst1 = paux.tile([1, c.S], F32, tag="aux", name="pst1")
            pst2 = paux.tile([1, c.S], F32, tag="aux", name="pst2")
            sq = []
            for dt in range(c.DT):
                s = sqp.tile([P, c.S], BF16, tag="sq", name="sq")
                nc.scalar.activation(s[:], f32(u_t[:, dt * c.S:(dt + 1) * c.S]),
                                     AF.Square)
                sq.append(s)
            for dt in range(c.DT):
                nc.tensor.matmul(pst1[:], ones_col[:, 0:1],
                                 u_t[:, dt * c.S:(dt + 1) * c.S],
                                 start=(dt == 0), stop=(dt == c.DT - 1),
                                 skip_group_check=True)
            for dt in range(c.DT):
                nc.tensor.matmul(pst2[:], onesb_col[:, 0:1], sq[dt][:],
                                 start=(dt == 0), stop=(dt == c.DT - 1),
                                 skip_group_check=True)
            # m2 = (S1/D)^2 ; v2 = S2/D - m2 ; A = exp(-.5 ln(v2+eps))
            m2 = rows.tile([1, c.S], F32, tag="row", name="m2")
            nc.scalar.activation(m2[:], pst1[:], AF.Square, scale=1.0 / c.D)
            v2 = rows.tile([1, c.S], F32, tag="row", name="v2")
            nc.vector.scalar_tensor_tensor(
                v2[:], pst2[:], 1.0 / c.D, m2[:], op0=ALU.mult, op1=ALU.subtract)
            lv = rows.tile([1, c.S], F32, tag="row", name="lv")
            nc.scalar.activation(lv[:], v2[:], AF.Ln, bias=eps_t[:])
            a_row = lnr.tile([1, c.S], F32R, tag="lnr", name="a_row")
            nc.scalar.activation(a_row[:], lv[:], AF.Exp, scale=-0.5)
            b_row = lnr.tile([1, c.S], F32R, tag="lnr", name="b_row")
            nc.vector.scalar_tensor_tensor(
                b_row[:], pst1[:], -1.0 / c.D, f32(a_row[:]),
                op0=ALU.mult, op1=ALU.mult)
            return a_row, b_row

        def ln_apply(u_t, a_row, b_row, ln_t, gb_off, cast_pool=None,
                     cast_tag=None):
            """x = u*A + B in place on the wide tile; optionally emit one wide
            bf16 copy of the result (for matmul moving operands)."""
            xb_t = None
            for dt in range(c.DT):
                us = u_t[:, dt * c.S:(dt + 1) * c.S]
                if trivial_affine:
                    if dt == 0:
                        pra = paux.tile([P, c.S], F32, tag="aux", name="pra")
                        prb = paux.tile([P, c.S], F32, tag="aux", name="prb")
                        nc.tensor.matmul(pra[:], ones_row[0:1, 0:P], a_row[:],
                                         start=True, stop=True)
                        nc.tensor.matmul(prb[:], ones_row[0:1, 0:P], b_row[:],
                                         start=True, stop=True)
                else:
                    pra = paux.tile([P, c.S], F32, tag="aux", name="pra")
                    prb = paux.tile([P, c.S], F32, tag="aux", name="prb")
                    gr = ln_t[0:1, gb_off + dt * P:gb_off + (dt + 1) * P]
                    br = ln_t[0:1, gb_off + c.D + dt * P:gb_off + c.D + (dt + 1) * P]
                    nc.tensor.matmul(pra[:], gr, a_row[:], start=True, stop=True)
                    nc.tensor.matmul(prb[:], gr, b_row[:], start=True, stop=False,
                                     skip_group_check=True)
                    nc.tensor.matmul(prb[:], br, ones_row[:, 0:c.S], start=False,
                                     stop=True, skip_group_check=True)
                t = tmpp.tile([P, c.S], F32, tag="tmp", name="tmp")
                nc.vector.tensor_tensor(t[:], f32(us), pra[:], op=ALU.mult)
                nc.vector.tensor_tensor(us, t[:], prb[:], op=ALU.add)
                if cast_pool is not None:
                    if xb_t is None:
                        xb_t = cast_pool.tile([P, c.DT * c.S], BF16, tag=cast_tag,
                                              name=cast_tag)
                    nc.vector.tensor_copy(out=xb_t[:, dt * c.S:(dt + 1) * c.S],
                                          in_=f32(us))
            return xb_t

        def s_proj(l, b):
            W = Wt[l]
            tok = slice(b * c.S, (b + 1) * c.S)
            xbt = xb_pre.pop(b, None)
            if xbt is None:
                xu = xt_w[b]
                xbt = xbp.tile([P, c.DT * c.S], BF16, tag="xb", name="xb")
                nc.vector.tensor_copy(out=xbt[:], in_=f32(xu[:]))
            xb = [xbt[:, dt * c.S:(dt + 1) * c.S] for dt in range(c.DT)]
            kq_l, stg_l = [], []
            for e in range(c.DT):
                pm = pmm.tile([P, c.S], F32, tag="mm", name="pm")
                for dt in range(c.DT):
                    nc.tensor.matmul(pm[:], W["wk"][dt][:, e * P:(e + 1) * P], xb[dt],
                                     start=(dt == 0), stop=(dt == c.DT - 1))
                kq = kqp.tile([P, c.S], BF16, tag=f"kq{e}", name=f"kq{e}")
                nc.scalar.activation(kq[:], pm[:], AF.Identity, bias=W["bk"][:, e:e + 1])
                kq_l.append(kq)
                st = stgp.tile([P, c.S], BF16, tag=f"stg{e}", name=f"stg{e}")
                nc.vector.scalar_tensor_tensor(
                    st[:], pm[:], W["bk"][:, e:e + 1], crow[:, tok],
                    op0=ALU.add, op1=ALU.mult)
                stg_l.append(st)
            vpl_l = []
            par = (l * c.Bl + b) % 2
            for tt in range(c.SB):
                pm = pmm.tile([P, c.D], F32, tag="mm", name="pm")
                for dt in range(c.DT):
                    nc.tensor.matmul(pm[:], y_sb[dt][b][:, tt * P:(tt + 1) * P],
                                     W["wv"][dt][:],
                                     start=(dt == 0), stop=(dt == c.DT - 1))
                vt = vt_pers[tt][par]
                nc.scalar.copy(out=vt[:, :, 0:DK],
                               in_=pm[:].rearrange("p (h k) -> p h k", h=c.H))
                vpl_l.append(vt)
            kq_of[l, b], stg_of[l, b], vpl_of[l, b] = kq_l, stg_l, vpl_l

        # score-tile plan: (entries, exp_width); entries = (kj, dst0, i0, w).
        # kj2+kj3 share one PSUM tile/exp (256+128 cols) to cut ACT op count.
        _ATT_PLAN = [
            ([(0, 0, 0, 512)], 512),
            ([(1, 0, 128, 384)], 384),
            ([(2, 0, 256, 256), (3, 256, 384, 128)], 384),
        ]

        def s_attn(l, b):
            kq_l, stg_l, vpl_l = kq_of.pop((l, b)), stg_of.pop((l, b)), vpl_of.pop((l, b))
            ctx_sb = [ctxp.tile([P, c.S], BF16, tag=f"ctx{dt}", name=f"ctx{dt}")
                      for dt in range(c.DT)]
            rr_p, pc_p = None, None
            for h in range(c.H):
                et, po = h // 2, (h % 2) * DK
                pc = pctx.tile([DK + 1, c.S], F32, tag="ctx_ps", name="pc")
                n_ent = sum(len(e) for e, _ in _ATT_PLAN)
                ei = 0
                for entries, we in _ATT_PLAN:
                    pst_ = psc.tile([P, c.S], F32, tag="sc", name="pst_")
                    for kj, dst0, i0, w in entries:
                        nc.tensor.matmul(
                            pst_[:, dst0:dst0 + w],
                            kq_l[et][po:po + DK, kj * P:(kj + 1) * P],
                            stg_l[et][po:po + DK, i0:i0 + w],
                            start=True, stop=True, skip_group_check=True)
                    pe_ = ptp.tile([P, c.S], BF16, tag="pt", name="pe_")
                    nc.scalar.activation(pe_[:, 0:we], pst_[:, 0:we], AF.Exp)
                    for kj, dst0, i0, w in entries:
                        nc.gpsimd.tensor_tensor(
                            pe_[:, dst0:dst0 + P], pe_[:, dst0:dst0 + P],
                            mtri_sb[:], op=ALU.mult)
                    for kj, dst0, i0, w in entries:
                        nc.tensor.matmul(pc[:, i0:i0 + w], vpl_l[kj][:, h, :],
                                         pe_[:, dst0:dst0 + w],
                                         start=(ei == 0), stop=(ei == n_ent - 1),
                                         skip_group_check=True)
                        ei += 1
                # rowsum -> reciprocal row (bf16) via exp(-ln(x)): stays in
                # the natural_log_exp table set (no ACT table switching)
                lr = rows.tile([1, c.S], F32, tag="row", name="lr")
                nc.scalar.activation(lr[:], pc[DK:DK + 1, :], AF.Ln)
                rr = rowb.tile([1, c.S], BF16, tag="rowb", name="rr")
                nc.scalar.activation(rr[:], lr[:], AF.Exp, scale=-1.0)
                if h % 2 == 0:
                    rr_p, pc_p = rr, pc
                else:
                    # normalize the head pair into ctx_sb[dt]
                    dt = h // 2
                    prb4 = paux.tile([P, c.S], F32, tag="aux", name="prb4")
                    nc.tensor.matmul(prb4[0:DK, :], ones_row_b[0:1, 0:DK],
                                     rr_p[:], start=True, stop=True)
                    nc.tensor.matmul(prb4[DK:P, :], ones_row_b[0:1, 0:DK],
                                     rr[:], start=True, stop=True,
                                     tile_position=(0, 64), skip_group_check=True)
                    rb = rbp.tile([P, c.S], F32, tag="rb", name="rb")
                    nc.vector.tensor_copy(out=rb[:], in_=prb4[:])
                    nc.vector.tensor_tensor(
                        ctx_sb[dt][0:DK, :], pc_p[0:DK, :], rb[0:DK, :],
                        op=ALU.mult)
                    nc.vector.tensor_tensor(
                        ctx_sb[dt][DK:P, :], pc[0:DK, :], rb[DK:P, :],
                        op=ALU.mult)
                    # zero_pad: first query column of the sequence
                    nc.scalar.copy(out=ctx_sb[dt][:, 0:1], in_=zeros_f[:, 0:1])
            ctx_of[l, b] = ctx_sb

        def s_outs(l, b):
            W = Wt[l]
            ctx_sb = ctx_of.pop((l, b))
            u_t = up.tile([P, c.DT * c.S], F32R, tag="u", name="u")
            for e in range(c.DT):
                pm = pmm.tile([P, c.S], F32, tag="mm", name="pm")
                for dt in range(c.DT):
                    nc.tensor.matmul(pm[:], W["wo"][dt][:, e * P:(e + 1) * P],
                                     ctx_sb[dt][:],
                                     start=(dt == 0), stop=(dt == c.DT - 1))
                nc.vector.scalar_tensor_tensor(
                    u_t[:, e * c.S:(e + 1) * c.S], pm[:], W["bo2"][:, e:e + 1],
                    f32(xt_w[b][:, e * c.S:(e + 1) * c.S]),
                    op0=ALU.add, op1=ALU.add)
            u_of[l, b] = u_t
            ln_of[l, b, 1] = ln_stats(u_t)

        def s_ln1ap(l, b):
            W = Wt[l]
            u_t = u_of[l, b]
            a_row, b_row = ln_of.pop((l, b, 1))
            x1b_of[l, b] = ln_apply(u_t, a_row, b_row, W.get("ln"), 0,
                                    cast_pool=x1bp, cast_tag="x1b")

        def s_ffn(l, b):
            W = Wt[l]
            u_t, x1b = u_of.pop((l, b)), x1b_of.pop((l, b))
            h_sb = []
            for ft in range(c.FT):
                pm = pmm.tile([P, c.S], F32, tag="mm", name="pm")
                for dt in range(c.DT):
                    nc.tensor.matmul(pm[:], W["w1"][dt][:, ft * P:(ft + 1) * P],
                                     x1b[:, dt * c.S:(dt + 1) * c.S],
                                     start=(dt == 0), stop=(dt == c.DT - 1))
                ht = hp.tile([P, c.S], BF16, tag="h", name="ht")
                if ft % 2 == 0:
                    nc.scalar.activation(ht[:], pm[:], AF.Relu,
                                         bias=W["b1"][:, ft:ft + 1])
                else:
                    nc.vector.tensor_scalar(ht[:], pm[:], W["b1"][:, ft:ft + 1],
                                            0.0, op0=ALU.add, op1=ALU.max)
                h_sb.append(ht)
            if b == c.Bl - 1 and l + 1 < c.L:
                load_w(l + 1, ["w1"])  # w1 tag slots just freed
            u2_t = xp.tile([P, c.DT * c.S], F32R, tag="x", name="x")
            for dt in range(c.DT):
                pm = pmm.tile([P, c.S], F32, tag="mm", name="pm")
                for ft in range(c.FT):
                    nc.tensor.matmul(pm[:], W["w2"][ft][:, dt * P:(dt + 1) * P],
                                     h_sb[ft][:],
                                     start=(ft == 0), stop=(ft == c.FT - 1))
                nc.vector.scalar_tensor_tensor(
                    u2_t[:, dt * c.S:(dt + 1) * c.S], pm[:], W["b2"][:, dt:dt + 1],
                    f32(u_t[:, dt * c.S:(dt + 1) * c.S]),
                    op0=ALU.add, op1=ALU.add)
            if b == c.Bl - 1 and l + 1 < c.L:
                load_w(l + 1, ["w2"])  # w2 tag slots just freed
            u_of[l, b, 2] = u2_t
            ln_of[l, b, 2] = ln_stats(u2_t)

        def s_ln2ap(l, b):
            W = Wt[l]
            tok = slice(b * c.S, (b + 1) * c.S)
            u2_t = u_of.pop((l, b, 2))
            a_row, b_row = ln_of.pop((l, b, 2))
            ln_apply(u2_t, a_row, b_row, W.get("ln"), 2 * c.D)
            if l + 1 < c.L:
                t = xbp.tile([P, c.DT * c.S], BF16, tag="xb", name="xb")
                nc.vector.tensor_copy(out=t[:], in_=f32(u2_t[:]))
                xb_pre[b] = t
            if l == c.L - 1:
                for dt in range(c.DT):
                    nc.sync.dma_start(
                        out=xoT[dt * P:(dt + 1) * P, tok],
                        in_=f32(u2_t[:, dt * c.S:(dt + 1) * c.S]))
            else:
                xt_w[b] = u2_t

        stages = [s_proj, s_attn, s_outs, s_ln1ap, s_ffn, s_ln2ap]
        LEAD = 5  # layer l+1's s_proj(b) lands one gstep after s_ln2ap(l, b)
        for g in range(LEAD * (c.L - 1) + 5 + c.Bl):
            for l in range(c.L):
                for si in (5, 3, 1, 2, 0, 4):
                    b = g - LEAD * l - si
                    if 0 <= b < c.Bl:
                        stages[si](l, b)
                        if b == c.Bl - 1 and l + 1 < c.L:
                            if si == 0:
                                load_w(l + 1, ["wk", "wv", "bias"])
                            elif si == 2:
                                load_w(l + 1, ["wo"])

    return nc


# ======================= host-side pre/post ==========================

def host_prep(inputs: dict, n_cores: int):
    """Full inputs -> (cfg, list of per-core in_maps, trivial_affine)."""
    import ml_dtypes

    q = np.ascontiguousarray(np.asarray(inputs["q_embed_data"], dtype=np.float32))
    qa = np.ascontiguousarray(np.asarray(inputs["qa_embed_data"], dtype=np.float32))
    fr = np.asarray(inputs["forget_rate"], dtype=np.float32)
    pos = np.asarray(inputs["pos_emb"], dtype=np.float32)
    Wk = np.asarray(inputs["Wk"], dtype=np.float32)
    Wv = np.asarray(inputs["Wv"], dtype=np.float32)
    Wo = np.asarray(inputs["Wo"], dtype=np.float32)
    W1 = np.asarray(inputs["W1"], dtype=np.float32)
    W2 = np.asarray(inputs["W2"], dtype=np.float32)
    bk = np.asarray(inputs["bk"], dtype=np.float32)
    bv = np.asarray(inputs["bv"], dtype=np.float32)
    bo = np.asarray(inputs["bo"], dtype=np.float32)
    b1 = np.asarray(inputs["b1"], dtype=np.float32)
    b2 = np.asarray(inputs["b2"], dtype=np.float32)
    g1 = np.asarray(inputs["ln1_g"], dtype=np.float32)
    be1 = np.asarray(inputs["ln1_b"], dtype=np.float32)
    g2 = np.asarray(inputs["ln2_g"], dtype=np.float32)
    be2 = np.asarray(inputs["ln2_b"], dtype=np.float32)

    B, S, D = q.shape
    L, F = W1.shape[0], W1.shape[1]
    H = D // DK
    assert B % n_cores == 0
    Bl = B // n_cores
    cfg = Cfg(Bl, S, D, H, F, L)
    scale = 1.0 / math.sqrt(DK)

    x0 = q + pos  # (B,S,D)
    y0 = qa + pos
    cv = (fr[..., 0] * scale).astype(np.float32)  # (B,S)

    def cols(v, n):  # per-feature vec [L, n*128] -> [L, 128, n]
        return np.ascontiguousarray(v.reshape(L, n, P).transpose(0, 2, 1))

    bo2 = bo + np.einsum("led,ld->le", Wo, bv)
    shared = {
        "wkT": np.ascontiguousarray(Wk.transpose(0, 2, 1)).astype(ml_dtypes.bfloat16),
        "wvT": np.ascontiguousarray(Wv.transpose(0, 2, 1)).astype(ml_dtypes.bfloat16),
        "woT": np.ascontiguousarray(Wo.transpose(0, 2, 1)).astype(ml_dtypes.bfloat16),
        "w1T": np.ascontiguousarray(W1.transpose(0, 2, 1)).astype(ml_dtypes.bfloat16),
        "w2T": np.ascontiguousarray(W2.transpose(0, 2, 1)).astype(ml_dtypes.bfloat16),
        "bkc": cols(bk, cfg.DT),
        "bo2c": cols(bo2, cfg.DT),
        "b1c": cols(b1, cfg.FT),
        "b2c": cols(b2, cfg.DT),
        "lnrow": np.ascontiguousarray(
            np.concatenate([g1, be1, g2, be2], axis=1)[:, None, :]),
        "mtri": np.triu(np.ones((P, P), np.float32), 1).astype(ml_dtypes.bfloat16),
    }
    trivial_affine = bool(np.all(g1 == 1) and np.all(g2 == 1)
                          and not be1.any() and not be2.any())

    in_maps = []
    for core in range(n_cores):
        bs = slice(core * Bl, (core + 1) * Bl)
        m = dict(shared)
        m["xT"] = np.ascontiguousarray(x0[bs].reshape(Bl * S, D).T)
        m["yT"] = np.ascontiguousarray(y0[bs].reshape(Bl * S, D).T).astype(ml_dtypes.bfloat16)
        m["cvec"] = np.ascontiguousarray(cv[bs].reshape(1, Bl * S))
        in_maps.append(m)
    return cfg, in_maps, trivial_affine


def host_post(cfg: Cfg, results):
    outs = []
    for r in results:
        xo = r["xoT"]  # [D, T]
        outs.append(xo.T.reshape(cfg.Bl, cfg.S, cfg.D))
    return np.concatenate(outs, axis=0)


# ======================= numpy reference (for dev tests) =============

def ref_np(inputs: dict):
    """Mirror of reference.py in numpy float64, arbitrary dims."""
    q = np.asarray(inputs["q_embed_data"], np.float64)
    qa = np.asarray(inputs["qa_embed_data"], np.float64)
    fr = np.asarray(inputs["forget_rate"], np.float64)
    pos = np.asarray(inputs["pos_emb"], np.float64)
    B, S, D = q.shape
    L = np.asarray(inputs["Wk"]).shape[0]
    H = D // DK
    x = q + pos
    y = qa + pos
    scale = 1.0 / math.sqrt(DK)
    allowed = np.tril(np.ones((S, S), bool), k=-1)
    for l in range(L):
        Wk = np.asarray(inputs["Wk"][l], np.float64)
        Wv = np.asarray(inputs["Wv"][l], np.float64)
        Wo = np.asarray(inputs["Wo"][l], np.float64)
        W1 = np.asarray(inputs["W1"][l], np.float64)
        W2 = np.asarray(inputs["W2"][l], np.float64)
        bk = np.asarray(inputs["bk"][l], np.float64)
        bv = np.asarray(inputs["bv"][l], np.float64)
        bo = np.asarray(inputs["bo"][l], np.float64)
        b1 = np.asarray(inputs["b1"][l], np.float64)
        b2 = np.asarray(inputs["b2"][l], np.float64)
        g1 = np.asarray(inputs["ln1_g"][l], np.float64)
        be1 = np.asarray(inputs["ln1_b"][l], np.float64)
        g2 = np.asarray(inputs["ln2_g"][l], np.float64)
        be2 = np.asarray(inputs["ln2_b"][l], np.float64)

        kq = (x @ Wk.T + bk).reshape(B, S, H, DK).transpose(0, 2, 1, 3)
        v = (y @ Wv.T + bv).reshape(B, S, H, DK).transpose(0, 2, 1, 3)
        sc = np.einsum("bhsd,bhtd->bhst", kq, kq) * scale
        sc = sc * fr[:, None, :, :]
        sc = np.where(allowed, sc, -np.inf)
        m = sc.max(axis=-1, keepdims=True)
        m = np.where(np.isfinite(m), m, 0.0)
        e = np.exp(sc - m)
        attn = e / e.sum(axis=-1, keepdims=True).clip(1e-300)
        attn[:, :, 0, :] = 0.0
        ctx = np.einsum("bhst,bhtd->bhsd", attn, v).transpose(0, 2, 1, 3).reshape(B, S, D)
        out = ctx @ Wo.T + bo

        def ln(t, g, bb):
            mu = t.mean(-1, keepdims=True)
            va = ((t - mu) ** 2).mean(-1, keepdims=True)
            return (t - mu) / np.sqrt(va + EPS) * g + bb

        x = ln(x + out, g1, be1)
        ff = np.maximum(x @ W1.T + b1, 0.0) @ W2.T + b2
        x = ln(x + ff, g2, be2)
    return x


# ======================= public entry point ==========================

N_CORES = 8
_nc_cache = {}


def kernel(**inputs) -> np.ndarray:
    import os
    from concourse.bass_utils import run_bass_kernel_spmd

    trace = bool(int(os.environ.get("KT_TRACE", "0")))
    if trace:
        _install_ntff_hook()
    cfg, in_maps, trivial = host_prep(inputs, N_CORES)
    key = (tuple(sorted(cfg.__dict__.items())), trivial)
    if key not in _nc_cache:
        _nc_cache[key] = build(cfg, trivial)
    res = run_bass_kernel_spmd(
        _nc_cache[key], in_maps, core_ids=list(range(N_CORES)), trace=trace
    )
    if res.exec_time_ns is not None:
        print(f"HW exec time: {res.exec_time_ns} ns")
        if res.instructions_and_trace is not None:
            print(f"trace: {res.instructions_and_trace[1]}")
    return host_post(cfg, res.results).astype(np.float32)
